# revision 1
# baseline (speedup 1.0000x reference)
"""DoRA adapter forward kernel for 8 trn2 NeuronCores — loop-minimized.

The repeat-slope cost on this rig is ~47us per STATIC instruction (NEFF
size), while executed instructions and HBM bytes are comparatively free
(see probe.py). So this kernel expresses everything in tc.For_i hardware
loops: ~200 static instructions instead of the ~6700 of the unrolled
baseline.

Math:  dora = dora_B @ dora_A                       [OUT, IN]
       num  = weight + ALPHA * dora                 [OUT, IN]
       s    = m / sqrt(colsum_over_out(num^2))      [1, IN]
       out  = x @ (num * s)^T + bias

Sharding (4x2 grid): 4-way split of the 8192 x-rows, 2-way split of OUT.

Per core:
  setup:  dora_A cast+scale, dora_B^T via PE transpose (For_i over 16
          o-tiles), bias row, m row.
  phase B (For_i over 32 i-tiles): load W[0:2048, i-tile] as a single
          rearranged DMA [128, 16*128]; 16 PE transposes -> ps_w
          [128i, 2048o]; rank-16 dora matmuls -> ps_d; DVE add ->
          numT (SBUF-resident bf16 [128, 32*2048], 128KB/partition);
          fused square+reduce (tensor_tensor_reduce) -> sumsq column.
  norm:   sumsq -> DRAM -> 8-way AllReduce (16KB) -> s = m/sqrt(.)
          (scale 1/MG since each o-half partial appears MG times);
          s folded into numT in-place (For_i over 32 i-tiles).
  phase D (For_i over 16 m-tiles): load x [128, 4096] f32; 32 PE
          transposes (2 psum groups) -> xT bf16; GEMM with bias seeded
          into PSUM via K=1 ones-matmul; ACT evac; store.

All HWDGE DMAs stay on the single nc.sync ring (mixed rings race on
trn2); SWDGE (gpsimd) only for the tiny sumsq scatter + collective.
"""

import sys

if "/opt/trn_rl_repo" not in sys.path:
    sys.path.insert(0, "/opt/trn_rl_repo")

from contextlib import ExitStack

import numpy as np

import concourse.bass as bass
import concourse.mybir as mybir
import concourse.tile as tile
from concourse import bacc
from concourse.bass import ds, ts
from concourse.bass_utils import run_bass_kernel_spmd
from concourse.masks import make_identity
from concourse.tile_rust import add_dep_helper

F32 = mybir.dt.float32
BF16 = mybir.dt.bfloat16

ALPHA = 16.0
N_CORES = 8
MG, OG = 4, 2

B_, S_, IN_FULL, OUT_FULL, R_ = 4, 2048, 4096, 4096, 16
M_FULL = B_ * S_
M_C = M_FULL // MG      # 2048 x-rows per core
O_C = OUT_FULL // OG    # 2048 out-cols per core

N_IT = IN_FULL // 128   # 32 i-tiles
N_OT = O_C // 128       # 16 o-tiles
N_MT = M_C // 128       # 16 m-tiles

GEMM_UNROLL = 8         # i-tiles per inner-loop body in phase D

BISECT = ""             # "", "nocc", "nod", "nob" (hang isolation)


def build_kernel(reps=1):
    nc = bacc.Bacc("TRN2", target_bir_lowering=False, debug=False,
                   num_devices=N_CORES)

    x_in = nc.dram_tensor("x_slice", [M_C, IN_FULL], F32, kind="ExternalInput")
    w_own = nc.dram_tensor("w_own", [O_C, IN_FULL], F32, kind="ExternalInput")
    bias_in = nc.dram_tensor("bias_own", [1, O_C], F32, kind="ExternalInput")
    m_in = nc.dram_tensor("m_row", [1, IN_FULL], F32, kind="ExternalInput")
    a_in = nc.dram_tensor("dora_a", [R_, IN_FULL], F32, kind="ExternalInput")
    b_own = nc.dram_tensor("dora_b_own", [O_C, R_], F32, kind="ExternalInput")
    out_t = nc.dram_tensor("out_slice", [M_C, O_C], F32, kind="ExternalOutput")

    s_dram = nc.dram_tensor("s_dram", [128, N_IT], F32)
    cc_out = nc.dram_tensor("cc_out", [128, N_IT], F32, addr_space="Shared")

    v = dict(locals())
    with tile.TileContext(nc) as tc:
        for _rep in range(reps):
            with ExitStack() as ctx:
                _emit(ctx, tc, v)
    nc.compile()
    return nc


def _emit(ctx, tc, v):
    nc = v["nc"]
    x_in, w_own, bias_in = v["x_in"], v["w_own"], v["bias_in"]
    m_in, a_in, b_own = v["m_in"], v["a_in"], v["b_own"]
    out_t, s_dram, cc_out = v["out_t"], v["s_dram"], v["cc_out"]

    const = ctx.enter_context(tc.tile_pool(name="const", bufs=1))

    ident = const.tile([128, 128], F32, tag="ident")
    make_identity(nc, ident[:])
    ones_row = const.tile([1, 128], BF16, tag="ones_row")
    nc.gpsimd.memset(ones_row[:], 1.0)

    # dora_A: f32 -> bf16 cast (SWDGE), pre-scaled by ALPHA
    a_raw = const.tile([R_, IN_FULL], BF16, tag="a_raw")
    nc.gpsimd.dma_start(out=a_raw[:], in_=a_in[:, :])
    a_bf = const.tile([R_, IN_FULL], BF16, tag="a_bf")
    nc.vector.tensor_scalar_mul(a_bf[:], a_raw[:], ALPHA)

    # dora_B^T: [R, O_C] bf16 via PE transpose, For_i over 16 o-tiles
    bt_bf = const.tile([R_, O_C], BF16, tag="bt_bf")
    with tc.tile_pool(name="btmp", bufs=2) as btmp, \
         tc.tile_pool(name="bt_ps", bufs=2, space="PSUM") as bt_ps:
        with tc.For_i(0, N_OT) as ot:
            b_t = btmp.tile([128, R_], F32, tag="b_t")
            nc.sync.dma_start(out=b_t[:], in_=b_own[ts(ot, 128), :])
            ps = bt_ps.tile([R_, 128], F32, tag="ps_bt")
            nc.tensor.transpose(ps[:], b_t[:], ident[:])
            nc.vector.tensor_copy(out=bt_bf[:, ts(ot, 128)], in_=ps[:])

    # bias row (bf16, used as K=1 matmul rhs to seed PSUM)
    bias_sb = const.tile([1, O_C], BF16, tag="bias_sb")
    nc.gpsimd.dma_start(out=bias_sb[0:1, :], in_=bias_in[:, :])

    # m in [128, N_IT] partition-major layout (p = i % 128, col = i // 128)
    m_t = const.tile([128, N_IT], F32, tag="m_t")
    nc.sync.dma_start(
        out=m_t[:], in_=m_in.ap().rearrange("a (c p) -> (a p) c", p=128))

    # numT: SBUF-resident [128, N_IT*2048] bf16 (128KB/partition)
    numT = const.tile([128, N_IT * O_C], BF16, tag="numT")
    ssq = const.tile([128, N_IT], F32, tag="ssq")

    if BISECT == "v0":
        return

    # ---------------- phase B: numT + sumsq ----------------
    n_it_b = 1 if BISECT == "nob" else N_IT
    if BISECT == "nob":
        nc.gpsimd.memset(ssq[:], 1.0)
    with tc.tile_pool(name="pB", bufs=2) as pB, \
         tc.tile_pool(name="pB_ps", bufs=1, space="PSUM") as pB_ps:
        with tc.For_i(0, n_it_b) as it:
            i0 = it * 128
            # W[0:2048, i0:i0+128] -> [128, 16*128]: col-block t holds
            # W[t*128+p, i0+c]
            w_sb = pB.tile([128, N_OT * 128], F32, tag="w_sb")
            nc.sync.dma_start(
                out=w_sb[:].rearrange("p (t i) -> p t i", t=N_OT),
                in_=w_own[:, ds(i0, 128)].rearrange("(t p) i -> p t i", p=128))
            nt_sl = numT[:, ts(it, O_C)]
            if BISECT == "b1":
                nc.vector.tensor_copy(out=nt_sl, in_=w_sb[:])
            else:
                # PSUM start=True zeroes the containing 2KB region, so only
                # the first transpose of each 512-f32 region starts; the
                # rest accumulate onto the zeroed bytes.
                ps_w = pB_ps.tile([128, O_C], F32, tag="ps_w")
                for ot in range(N_OT):
                    nc.tensor.matmul(
                        ps_w[:, ot * 128:(ot + 1) * 128],
                        lhsT=w_sb[:, ot * 128:(ot + 1) * 128], rhs=ident[:],
                        is_transpose=True, start=(ot % 4 == 0),
                        stop=(BISECT in ("b2", "b3") and ot % 4 == 3),
                        skip_group_check=True)
                if BISECT == "b2":
                    nc.vector.tensor_copy(out=nt_sl, in_=ps_w[:])
                elif BISECT == "b3":
                    a_cur = pB.tile([R_, 128], BF16, tag="a_cur")
                    nc.vector.tensor_copy(
                        out=a_cur[:], in_=a_bf[:, ds(i0, 128)])
                    ps_d = pB_ps.tile([128, 512], F32, tag="ps_d")
                    for q in range(O_C // 512):
                        nc.tensor.matmul(
                            ps_d[:], lhsT=a_cur[:],
                            rhs=bt_bf[:, q * 512:(q + 1) * 512],
                            start=True, stop=True)
                    nc.vector.tensor_copy(out=nt_sl, in_=ps_w[:])
                elif BISECT == "b4":
                    a_cur = pB.tile([R_, 128], BF16, tag="a_cur")
                    nc.vector.tensor_copy(
                        out=a_cur[:], in_=a_bf[:, ds(i0, 128)])
                    for q in range(O_C // 512):
                        nc.tensor.matmul(
                            ps_w[:, q * 512:(q + 1) * 512],
                            lhsT=a_cur[:],
                            rhs=bt_bf[:, q * 512:(q + 1) * 512],
                            start=False, stop=True, skip_group_check=True)
                    nc.vector.tensor_copy(out=nt_sl, in_=ps_w[:])
                else:
                    # lhsT must be at a static address (no register offsets
                    # in ldweights) -> stage the current dora_A slice; the
                    # dora matmuls accumulate on the transposed W in PSUM
                    a_cur = pB.tile([R_, 128], BF16, tag="a_cur")
                    nc.vector.tensor_copy(
                        out=a_cur[:], in_=a_bf[:, ds(i0, 128)])
                    for q in range(O_C // 512):
                        nc.tensor.matmul(
                            ps_w[:, q * 512:(q + 1) * 512],
                            lhsT=a_cur[:],
                            rhs=bt_bf[:, q * 512:(q + 1) * 512],
                            start=False, stop=True, skip_group_check=True)
                    nc.vector.tensor_copy(out=nt_sl, in_=ps_w[:])
                    # sumsq via ACT Square + accum (all-static APs: a
                    # dynamic-AP tensor_tensor_reduce faults the exec unit),
                    # then a dynamic-out copy into this i-tile's column
                    sq = pB.tile([128, O_C], BF16, tag="sq")
                    ssq_tmp = pB.tile([128, 1], F32, tag="ssq_tmp")
                    nc.scalar.activation(
                        sq[:], ps_w[:], mybir.ActivationFunctionType.Square,
                        0.0, 1.0, accum_out=ssq_tmp[:])
                    nc.vector.tensor_copy(
                        out=ssq[:, ds(it, 1)], in_=ssq_tmp[:])

    if BISECT in ("v1", "b1", "b2", "b3", "b4"):
        return

    # ---------------- norm: AllReduce sumsq -> s, fold into numT ----------
    s_raw = const.tile([128, N_IT], F32, tag="s_raw")
    if BISECT in ("nocc", "v2", "v3"):
        nc.vector.tensor_scalar_mul(s_raw[:], ssq[:], float(OG))
    else:
        st = nc.gpsimd.dma_start(out=s_dram.ap(), in_=ssq[:])
        cc = nc.gpsimd.collective_compute(
            "AllReduce", mybir.AluOpType.add,
            ins=[s_dram.ap()], outs=[cc_out.ap()],
            replica_groups=[list(range(N_CORES))])
        add_dep_helper(cc.ins, st.ins, reason="cc RAW on s_dram")
        ld = nc.sync.dma_start(out=s_raw[:], in_=cc_out.ap())
        add_dep_helper(ld.ins, cc.ins, reason="s_raw RAW on collective out")
    s_sq = const.tile([128, N_IT], F32, tag="s_sq")
    # each o-half partial is contributed by MG cores -> reduce = MG * full
    nc.scalar.activation(s_sq[:], s_raw[:],
                         mybir.ActivationFunctionType.Sqrt, 0.0, 1.0 / MG)
    s_rc = const.tile([128, N_IT], F32, tag="s_rc")
    nc.vector.reciprocal(s_rc[:], s_sq[:])
    s_t = const.tile([128, N_IT], F32, tag="s_t")
    nc.vector.tensor_mul(out=s_t[:], in0=s_rc[:], in1=m_t[:])

    if BISECT == "v2":
        return

    with tc.For_i(0, N_IT) as it:
        nt_sl = numT[:, ts(it, O_C)]
        nc.vector.tensor_scalar_mul(nt_sl, nt_sl, s_t[:, ds(it, 1)])

    if BISECT == "v3":
        return

    # ---------------- phase D: out = xT^T @ numT + bias ----------------
    n_mt_d = 1 if BISECT == "nod" else N_MT
    with tc.tile_pool(name="pD", bufs=1) as pD, \
         tc.tile_pool(name="pD_ps", bufs=1, space="PSUM") as pD_ps, \
         tc.tile_pool(name="pD_ps2", bufs=1, space="PSUM") as pD_ps2:
        with tc.For_i(0, n_mt_d) as mt:
            x_sb = pD.tile([128, IN_FULL], F32, tag="x_sb")
            nc.sync.dma_start(out=x_sb[:], in_=x_in[ts(mt, 128), :])
            xt_sb = pD.tile([128, IN_FULL], BF16, tag="xt_sb")
            ps_x = pD_ps.tile([128, O_C], F32, tag="ps_x")
            for half in range(2):
                for j in range(16):
                    itt = half * 16 + j
                    nc.tensor.matmul(
                        ps_x[:, j * 128:(j + 1) * 128],
                        lhsT=x_sb[:, itt * 128:(itt + 1) * 128], rhs=ident[:],
                        is_transpose=True, start=(j % 4 == 0),
                        stop=(j % 4 == 3), skip_group_check=True)
                nc.scalar.copy(
                    out=xt_sb[:, half * O_C:(half + 1) * O_C], in_=ps_x[:])

            ps_o = pD_ps2.tile([128, O_C], F32, tag="ps_o")
            for q in range(O_C // 512):
                nc.tensor.matmul(
                    ps_o[:, q * 512:(q + 1) * 512],
                    lhsT=ones_row[:],
                    rhs=bias_sb[:, q * 512:(q + 1) * 512],
                    start=True, stop=False, skip_group_check=True)
            # i-loop static (lhsT needs static addresses); mt-loop dynamic
            for u in range(N_IT):
                last = u == N_IT - 1
                for q in range(O_C // 512):
                    nc.tensor.matmul(
                        ps_o[:, q * 512:(q + 1) * 512],
                        lhsT=xt_sb[:, u * 128:(u + 1) * 128],
                        rhs=numT[:, u * O_C + q * 512:u * O_C + (q + 1) * 512],
                        start=False, stop=last, skip_group_check=True)
            o_sb = pD.tile([128, O_C], F32, tag="o_sb")
            nc.scalar.copy(out=o_sb[:], in_=ps_o[:])
            nc.sync.dma_start(out=out_t[ts(mt, 128), :], in_=o_sb[:])


_NC_CACHE = {}


def get_nc(reps=1):
    if reps not in _NC_CACHE:
        _NC_CACHE[reps] = build_kernel(reps)
    return _NC_CACHE[reps]


def make_in_maps(x, weight, bias, m, dora_A, dora_B):
    x = np.ascontiguousarray(np.asarray(x, dtype=np.float32))
    weight = np.ascontiguousarray(np.asarray(weight, dtype=np.float32))
    bias = np.ascontiguousarray(np.asarray(bias, dtype=np.float32))
    m = np.ascontiguousarray(np.asarray(m, dtype=np.float32))
    dora_A = np.ascontiguousarray(np.asarray(dora_A, dtype=np.float32))
    dora_B = np.ascontiguousarray(np.asarray(dora_B, dtype=np.float32))
    xf = x.reshape(M_FULL, IN_FULL)
    in_maps = []
    for c in range(N_CORES):
        g, h = divmod(c, OG)
        o0 = h * O_C
        in_maps.append({
            "x_slice": np.ascontiguousarray(xf[g * M_C:(g + 1) * M_C]),
            "w_own": np.ascontiguousarray(weight[o0:o0 + O_C]),
            "bias_own": np.ascontiguousarray(bias[o0:o0 + O_C].reshape(1, O_C)),
            "m_row": np.ascontiguousarray(m.reshape(1, IN_FULL)),
            "dora_a": dora_A,
            "dora_b_own": np.ascontiguousarray(dora_B[o0:o0 + O_C]),
        })
    return in_maps


def kernel(x, weight, bias, m, dora_A, dora_B, _trace=False, _trace_kwargs=None):
    in_maps = make_in_maps(x, weight, bias, m, dora_A, dora_B)
    res = run_bass_kernel_spmd(
        get_nc(), in_maps, core_ids=list(range(N_CORES)),
        trace=_trace, **(_trace_kwargs or {}))
    out = np.empty((M_FULL, OUT_FULL), np.float32)
    for c in range(N_CORES):
        g, h = divmod(c, OG)
        out[g * M_C:(g + 1) * M_C, h * O_C:(h + 1) * O_C] = \
            res.results[c]["out_slice"]
    ret = out.reshape(B_, S_, OUT_FULL)
    if _trace:
        return ret, res
    return ret



# revision 3
# speedup vs baseline: 1215.1427x; 1215.1427x over previous
"""DoRA adapter forward kernel for 8 trn2 NeuronCores — loop-minimized.

The repeat-slope cost on this rig is ~47us per STATIC instruction (NEFF
size), while executed instructions and HBM bytes are comparatively free
(see probe.py). So this kernel expresses everything in tc.For_i hardware
loops: ~200 static instructions instead of the ~6700 of the unrolled
baseline.

Math:  dora = dora_B @ dora_A                       [OUT, IN]
       num  = weight + ALPHA * dora                 [OUT, IN]
       s    = m / sqrt(colsum_over_out(num^2))      [1, IN]
       out  = x @ (num * s)^T + bias

Sharding (4x2 grid): 4-way split of the 8192 x-rows, 2-way split of OUT.

Per core:
  setup:  dora_A cast+scale, dora_B^T via PE transpose (For_i over 16
          o-tiles), bias row, m row.
  phase B (For_i over 32 i-tiles): load W[0:2048, i-tile] as a single
          rearranged DMA [128, 16*128]; 16 PE transposes -> ps_w
          [128i, 2048o]; rank-16 dora matmuls -> ps_d; DVE add ->
          numT (SBUF-resident bf16 [128, 32*2048], 128KB/partition);
          fused square+reduce (tensor_tensor_reduce) -> sumsq column.
  norm:   sumsq -> DRAM -> 8-way AllReduce (16KB) -> s = m/sqrt(.)
          (scale 1/MG since each o-half partial appears MG times);
          s folded into numT in-place (For_i over 32 i-tiles).
  phase D (For_i over 16 m-tiles): load x [128, 4096] f32; 32 PE
          transposes (2 psum groups) -> xT bf16; GEMM with bias seeded
          into PSUM via K=1 ones-matmul; ACT evac; store.

All HWDGE DMAs stay on the single nc.sync ring (mixed rings race on
trn2); SWDGE (gpsimd) only for the tiny sumsq scatter + collective.
"""

import sys

if "/opt/trn_rl_repo" not in sys.path:
    sys.path.insert(0, "/opt/trn_rl_repo")

from contextlib import ExitStack

import numpy as np

import concourse.bass as bass
import concourse.mybir as mybir
import concourse.tile as tile
from concourse import bacc
from concourse.bass import ds, ts
from concourse.bass_utils import run_bass_kernel_spmd
from concourse.masks import make_identity
from concourse.tile_rust import add_dep_helper

F32 = mybir.dt.float32
BF16 = mybir.dt.bfloat16

ALPHA = 16.0
N_CORES = 8
MG, OG = 4, 2

B_, S_, IN_FULL, OUT_FULL, R_ = 4, 2048, 4096, 4096, 16
M_FULL = B_ * S_
M_C = M_FULL // MG      # 2048 x-rows per core
O_C = OUT_FULL // OG    # 2048 out-cols per core

N_IT = IN_FULL // 128   # 32 i-tiles
N_OT = O_C // 128       # 16 o-tiles
N_MT = M_C // 128       # 16 m-tiles

GEMM_UNROLL = 8         # i-tiles per inner-loop body in phase D

BISECT = ""             # "", "nocc", "nod", "nob" (hang isolation)
SQRT_FN = None          # tsim.py overrides with a range-safe ACT fn


def build_kernel(reps=1):
    nc = bacc.Bacc("TRN2", target_bir_lowering=False, debug=False,
                   num_devices=N_CORES)

    x_in = nc.dram_tensor("x_slice", [M_C, IN_FULL], F32, kind="ExternalInput")
    w_own = nc.dram_tensor("w_own", [O_C, IN_FULL], F32, kind="ExternalInput")
    bias_in = nc.dram_tensor("bias_own", [1, O_C], F32, kind="ExternalInput")
    m_in = nc.dram_tensor("m_row", [1, IN_FULL], F32, kind="ExternalInput")
    a_in = nc.dram_tensor("dora_a", [R_, IN_FULL], F32, kind="ExternalInput")
    b_own = nc.dram_tensor("dora_b_own", [O_C, R_], F32, kind="ExternalInput")
    out_t = nc.dram_tensor("out_slice", [M_C, O_C], F32, kind="ExternalOutput")

    s_dram = nc.dram_tensor("s_dram", [128, N_IT], F32)
    cc_out = nc.dram_tensor("cc_out", [128, N_IT], F32, addr_space="Shared")

    v = dict(locals())
    with tile.TileContext(nc) as tc:
        for _rep in range(reps):
            with ExitStack() as ctx:
                _emit(ctx, tc, v)
    nc.compile()
    return nc


def _emit(ctx, tc, v):
    nc = v["nc"]
    x_in, w_own, bias_in = v["x_in"], v["w_own"], v["bias_in"]
    m_in, a_in, b_own = v["m_in"], v["a_in"], v["b_own"]
    out_t, s_dram, cc_out = v["out_t"], v["s_dram"], v["cc_out"]

    const = ctx.enter_context(tc.tile_pool(name="const", bufs=1))

    ident = const.tile([128, 128], F32, tag="ident")
    make_identity(nc, ident[:])
    ones_row = const.tile([1, 128], BF16, tag="ones_row")
    nc.gpsimd.memset(ones_row[:], 1.0)

    # dora_A: f32 -> bf16 cast (SWDGE), pre-scaled by ALPHA
    a_raw = const.tile([R_, IN_FULL], BF16, tag="a_raw")
    nc.gpsimd.dma_start(out=a_raw[:], in_=a_in[:, :])
    a_bf = const.tile([R_, IN_FULL], BF16, tag="a_bf")
    nc.vector.tensor_scalar_mul(a_bf[:], a_raw[:], ALPHA)

    # dora_B^T: [R, O_C] bf16 via PE transpose, For_i over 16 o-tiles
    bt_bf = const.tile([R_, O_C], BF16, tag="bt_bf")
    with tc.tile_pool(name="btmp", bufs=2) as btmp, \
         tc.tile_pool(name="bt_ps", bufs=2, space="PSUM") as bt_ps:
        with tc.For_i(0, N_OT) as ot:
            b_t = btmp.tile([128, R_], F32, tag="b_t")
            nc.sync.dma_start(out=b_t[:], in_=b_own[ts(ot, 128), :])
            ps = bt_ps.tile([R_, 128], F32, tag="ps_bt")
            nc.tensor.transpose(ps[:], b_t[:], ident[:])
            nc.vector.tensor_copy(out=bt_bf[:, ts(ot, 128)], in_=ps[:])

    # bias row (bf16, used as K=1 matmul rhs to seed PSUM)
    bias_sb = const.tile([1, O_C], BF16, tag="bias_sb")
    nc.gpsimd.dma_start(out=bias_sb[0:1, :], in_=bias_in[:, :])

    # m in [128, N_IT] partition-major layout (p = i % 128, col = i // 128)
    m_t = const.tile([128, N_IT], F32, tag="m_t")
    nc.sync.dma_start(
        out=m_t[:], in_=m_in.ap().rearrange("a (c p) -> (a p) c", p=128))

    # numT: SBUF-resident [128, N_IT*2048] bf16 (128KB/partition)
    numT = const.tile([128, N_IT * O_C], BF16, tag="numT")
    ssq = const.tile([128, N_IT], F32, tag="ssq")

    if BISECT == "v0":
        return

    # ---------------- phase B: numT + sumsq ----------------
    n_it_b = 1 if BISECT == "nob" else N_IT
    if BISECT == "nob":
        nc.gpsimd.memset(ssq[:], 1.0)
    with tc.tile_pool(name="pB", bufs=2) as pB, \
         tc.tile_pool(name="pB_ps", bufs=1, space="PSUM") as pB_ps:
        with tc.For_i(0, n_it_b) as it:
            i0 = it * 128
            # W[0:2048, i0:i0+128] -> [128, 16*128]: col-block t holds
            # W[t*128+p, i0+c]
            w_sb = pB.tile([128, N_OT * 128], F32, tag="w_sb")
            nc.sync.dma_start(
                out=w_sb[:].rearrange("p (t i) -> p t i", t=N_OT),
                in_=w_own[:, ds(i0, 128)].rearrange("(t p) i -> p t i", p=128))
            nt_sl = numT[:, ts(it, O_C)]
            if BISECT == "b1":
                nc.vector.tensor_copy(out=nt_sl, in_=w_sb[:])
            else:
                # PSUM start=True zeroes the containing 2KB region, so only
                # the first transpose of each 512-f32 region starts; the
                # rest accumulate onto the zeroed bytes.
                ps_w = pB_ps.tile([128, O_C], F32, tag="ps_w")
                for ot in range(N_OT):
                    nc.tensor.matmul(
                        ps_w[:, ot * 128:(ot + 1) * 128],
                        lhsT=w_sb[:, ot * 128:(ot + 1) * 128], rhs=ident[:],
                        is_transpose=True, start=(ot % 4 == 0),
                        stop=(BISECT in ("b2", "b3") and ot % 4 == 3),
                        skip_group_check=True)
                if BISECT == "b2":
                    nc.vector.tensor_copy(out=nt_sl, in_=ps_w[:])
                elif BISECT == "b3":
                    a_cur = pB.tile([R_, 128], BF16, tag="a_cur")
                    nc.vector.tensor_copy(
                        out=a_cur[:], in_=a_bf[:, ds(i0, 128)])
                    ps_d = pB_ps.tile([128, 512], F32, tag="ps_d")
                    for q in range(O_C // 512):
                        nc.tensor.matmul(
                            ps_d[:], lhsT=a_cur[:],
                            rhs=bt_bf[:, q * 512:(q + 1) * 512],
                            start=True, stop=True)
                    nc.vector.tensor_copy(out=nt_sl, in_=ps_w[:])
                elif BISECT == "b4":
                    a_cur = pB.tile([R_, 128], BF16, tag="a_cur")
                    nc.vector.tensor_copy(
                        out=a_cur[:], in_=a_bf[:, ds(i0, 128)])
                    for q in range(O_C // 512):
                        nc.tensor.matmul(
                            ps_w[:, q * 512:(q + 1) * 512],
                            lhsT=a_cur[:],
                            rhs=bt_bf[:, q * 512:(q + 1) * 512],
                            start=False, stop=True, skip_group_check=True)
                    nc.vector.tensor_copy(out=nt_sl, in_=ps_w[:])
                else:
                    # lhsT must be at a static address (no register offsets
                    # in ldweights) -> stage the current dora_A slice; the
                    # dora matmuls accumulate on the transposed W in PSUM
                    a_cur = pB.tile([R_, 128], BF16, tag="a_cur")
                    nc.vector.tensor_copy(
                        out=a_cur[:], in_=a_bf[:, ds(i0, 128)])
                    for q in range(O_C // 512):
                        nc.tensor.matmul(
                            ps_w[:, q * 512:(q + 1) * 512],
                            lhsT=a_cur[:],
                            rhs=bt_bf[:, q * 512:(q + 1) * 512],
                            start=False, stop=True, skip_group_check=True)
                    nc.vector.tensor_copy(out=nt_sl, in_=ps_w[:])
                    # sumsq via ACT Square + accum (all-static APs: a
                    # dynamic-AP tensor_tensor_reduce faults the exec unit),
                    # then a dynamic-out copy into this i-tile's column
                    sq = pB.tile([128, O_C], BF16, tag="sq")
                    ssq_tmp = pB.tile([128, 1], F32, tag="ssq_tmp")
                    nc.scalar.activation(
                        sq[:], ps_w[:], mybir.ActivationFunctionType.Square,
                        0.0, 1.0, accum_out=ssq_tmp[:])
                    nc.vector.tensor_copy(
                        out=ssq[:, ds(it, 1)], in_=ssq_tmp[:])

    if BISECT in ("v1", "b1", "b2", "b3", "b4"):
        return

    # ---------------- norm: AllReduce sumsq -> s, fold into numT ----------
    s_raw = const.tile([128, N_IT], F32, tag="s_raw")
    if BISECT in ("nocc", "v2", "v3"):
        nc.vector.tensor_scalar_mul(s_raw[:], ssq[:], float(OG))
    else:
        st = nc.gpsimd.dma_start(out=s_dram.ap(), in_=ssq[:])
        cc = nc.gpsimd.collective_compute(
            "AllReduce", mybir.AluOpType.add,
            ins=[s_dram.ap()], outs=[cc_out.ap()],
            replica_groups=[list(range(N_CORES))])
        add_dep_helper(cc.ins, st.ins, reason="cc RAW on s_dram")
        ld = nc.sync.dma_start(out=s_raw[:], in_=cc_out.ap())
        add_dep_helper(ld.ins, cc.ins, reason="s_raw RAW on collective out")
    s_sq = const.tile([128, N_IT], F32, tag="s_sq")
    # each o-half partial is contributed by MG cores -> reduce = MG * full
    nc.scalar.activation(s_sq[:], s_raw[:],
                         SQRT_FN or mybir.ActivationFunctionType.Sqrt,
                         0.0, 1.0 / MG)
    s_rc = const.tile([128, N_IT], F32, tag="s_rc")
    nc.vector.reciprocal(s_rc[:], s_sq[:])
    s_t = const.tile([128, N_IT], F32, tag="s_t")
    nc.vector.tensor_mul(out=s_t[:], in0=s_rc[:], in1=m_t[:])

    if BISECT == "v2":
        return

    with tc.For_i(0, N_IT) as it:
        nt_sl = numT[:, ts(it, O_C)]
        nc.vector.tensor_scalar_mul(nt_sl, nt_sl, s_t[:, ds(it, 1)])

    if BISECT == "v3":
        return

    # ---------------- phase D: out = xT^T @ numT + bias ----------------
    n_mt_d = 1 if BISECT == "nod" else N_MT
    with tc.tile_pool(name="pD", bufs=1) as pD, \
         tc.tile_pool(name="pD_ps", bufs=1, space="PSUM") as pD_ps, \
         tc.tile_pool(name="pD_ps2", bufs=1, space="PSUM") as pD_ps2:
        with tc.For_i(0, n_mt_d) as mt:
            x_sb = pD.tile([128, IN_FULL], F32, tag="x_sb")
            nc.sync.dma_start(out=x_sb[:], in_=x_in[ts(mt, 128), :])
            xt_sb = pD.tile([128, IN_FULL], BF16, tag="xt_sb")
            ps_x = pD_ps.tile([128, O_C], F32, tag="ps_x")
            for half in range(2):
                for j in range(16):
                    itt = half * 16 + j
                    nc.tensor.matmul(
                        ps_x[:, j * 128:(j + 1) * 128],
                        lhsT=x_sb[:, itt * 128:(itt + 1) * 128], rhs=ident[:],
                        is_transpose=True, start=(j % 4 == 0),
                        stop=(j % 4 == 3), skip_group_check=True)
                nc.scalar.copy(
                    out=xt_sb[:, half * O_C:(half + 1) * O_C], in_=ps_x[:])

            ps_o = pD_ps2.tile([128, O_C], F32, tag="ps_o")
            for q in range(O_C // 512):
                nc.tensor.matmul(
                    ps_o[:, q * 512:(q + 1) * 512],
                    lhsT=ones_row[:],
                    rhs=bias_sb[:, q * 512:(q + 1) * 512],
                    start=True, stop=False, skip_group_check=True)
            # i-loop static (lhsT needs static addresses); mt-loop dynamic
            for u in range(N_IT):
                last = u == N_IT - 1
                for q in range(O_C // 512):
                    nc.tensor.matmul(
                        ps_o[:, q * 512:(q + 1) * 512],
                        lhsT=xt_sb[:, u * 128:(u + 1) * 128],
                        rhs=numT[:, u * O_C + q * 512:u * O_C + (q + 1) * 512],
                        start=False, stop=last, skip_group_check=True)
            o_sb = pD.tile([128, O_C], F32, tag="o_sb")
            nc.scalar.copy(out=o_sb[:], in_=ps_o[:])
            nc.sync.dma_start(out=out_t[ts(mt, 128), :], in_=o_sb[:])


_NC_CACHE = {}


def get_nc(reps=1):
    if reps not in _NC_CACHE:
        _NC_CACHE[reps] = build_kernel(reps)
    return _NC_CACHE[reps]


def make_in_maps(x, weight, bias, m, dora_A, dora_B):
    x = np.ascontiguousarray(np.asarray(x, dtype=np.float32))
    weight = np.ascontiguousarray(np.asarray(weight, dtype=np.float32))
    bias = np.ascontiguousarray(np.asarray(bias, dtype=np.float32))
    m = np.ascontiguousarray(np.asarray(m, dtype=np.float32))
    dora_A = np.ascontiguousarray(np.asarray(dora_A, dtype=np.float32))
    dora_B = np.ascontiguousarray(np.asarray(dora_B, dtype=np.float32))
    xf = x.reshape(M_FULL, IN_FULL)
    in_maps = []
    for c in range(N_CORES):
        g, h = divmod(c, OG)
        o0 = h * O_C
        in_maps.append({
            "x_slice": np.ascontiguousarray(xf[g * M_C:(g + 1) * M_C]),
            "w_own": np.ascontiguousarray(weight[o0:o0 + O_C]),
            "bias_own": np.ascontiguousarray(bias[o0:o0 + O_C].reshape(1, O_C)),
            "m_row": np.ascontiguousarray(m.reshape(1, IN_FULL)),
            "dora_a": dora_A,
            "dora_b_own": np.ascontiguousarray(dora_B[o0:o0 + O_C]),
        })
    return in_maps


def kernel(x, weight, bias, m, dora_A, dora_B, _trace=False, _trace_kwargs=None):
    in_maps = make_in_maps(x, weight, bias, m, dora_A, dora_B)
    res = run_bass_kernel_spmd(
        get_nc(), in_maps, core_ids=list(range(N_CORES)),
        trace=_trace, **(_trace_kwargs or {}))
    out = np.empty((M_FULL, OUT_FULL), np.float32)
    for c in range(N_CORES):
        g, h = divmod(c, OG)
        out[g * M_C:(g + 1) * M_C, h * O_C:(h + 1) * O_C] = \
            res.results[c]["out_slice"]
    ret = out.reshape(B_, S_, OUT_FULL)
    if _trace:
        return ret, res
    return ret

